# revision 1
# baseline (speedup 1.0000x reference)
"""LoRA Linear layer on 8 Trainium2 NeuronCores.

Computes out = x @ W.T + bias + scaling * (x @ A.T) @ B.T for
x [4, 4096, 4096] f32, W [4096, 4096], bias [4096], A [16, 4096], B [4096, 16].

Strategy:
- Host: fold the rank-16 LoRA path into the weight (exact up to f32
  rounding): W_eff = W.T + scaling * (A.T @ B.T), layout [in, out].
- Shard data-parallel over the batch: 16384 rows of x split 8 x 2048.
  W_eff/bias replicated per core; no collectives.
- Per core: out_s[2048, 4096] = x_s @ W_eff + bias as an fp16 matmul with
  fp32 PSUM accumulation (scale-relative absmax error ~3e-4 vs f32).
  x_s.T stays resident in SBUF ([128, 32, 2048] fp16 = 128KB/partition);
  W_eff streams through SBUF once per core in [128, 512] chunks.
"""

import numpy as np

IN_F = 4096
OUT_F = 4096
R = 16
SCALING = 32.0 / R
N_CORES = 8
M_TOTAL = 4 * 4096
M_CORE = M_TOTAL // N_CORES  # 2048

P = 128
KO = IN_F // P  # 32 contraction chunks
NW = 512  # n-tile width (one PSUM bank of f32)
NT = OUT_F // NW  # 8 n tiles
MPG = 8  # m-tiles per psum group (8 PSUM banks)
MG = M_CORE // (P * MPG)  # 2 m groups

_CACHE = {}


def _build_nc():
    import concourse.mybir as mybir
    import concourse.tile as tile
    from concourse import bacc

    nc = bacc.Bacc("TRN2", target_bir_lowering=False, debug=False,
                   num_devices=N_CORES)
    xT = nc.dram_tensor("xT", [IN_F, M_CORE], mybir.dt.float16,
                        kind="ExternalInput").ap()
    w = nc.dram_tensor("w", [IN_F, OUT_F], mybir.dt.float16,
                       kind="ExternalInput").ap()
    biasr = nc.dram_tensor("biasr", [P, OUT_F], mybir.dt.float32,
                           kind="ExternalInput").ap()
    out = nc.dram_tensor("out", [M_CORE, OUT_F], mybir.dt.float32,
                         kind="ExternalOutput").ap()

    xr = xT.rearrange("(ko p) m -> ko p m", p=P)
    wr = w.rearrange("(ko p) n -> ko p n", p=P)

    with tile.TileContext(nc) as tc:
        with (
            tc.tile_pool(name="xpool", bufs=1) as xpool,
            tc.tile_pool(name="wpool", bufs=36) as wpool,
            tc.tile_pool(name="bpool", bufs=1) as bpool,
            tc.tile_pool(name="opool", bufs=4) as opool,
            tc.tile_pool(name="pspool", bufs=1, space="PSUM") as pspool,
        ):
            bias_sb = bpool.tile([P, OUT_F], mybir.dt.float32, name="bias_sb")
            nc.sync.dma_start(bias_sb[:], biasr)

            # x chunks resident for the whole kernel; interleave the first
            # n-tile's weight chunks so compute can start immediately.
            x_sb = []
            w_sb = {}
            for ko in range(KO):
                xt = xpool.tile([P, M_CORE], mybir.dt.float16,
                                name=f"x{ko}", tag=f"x{ko}")
                nc.sync.dma_start(xt[:], xr[ko])
                x_sb.append(xt)
                wt = wpool.tile([P, NW], mybir.dt.float16,
                                name=f"w0_{ko}", tag="w", bufs=36)
                nc.sync.dma_start(wt[:], wr[ko, :, 0:NW])
                w_sb[(0, ko)] = wt

            for nt in range(NT):
                if nt > 0:
                    for ko in range(KO):
                        wt = wpool.tile([P, NW], mybir.dt.float16,
                                        name=f"w{nt}_{ko}", tag="w", bufs=36)
                        nc.sync.dma_start(wt[:], wr[ko, :, nt * NW:(nt + 1) * NW])
                        w_sb[(nt, ko)] = wt

                for mg in range(MG):
                    psums = [
                        pspool.tile([P, NW], mybir.dt.float32,
                                    name=f"ps_{nt}_{mg}_{mi}", tag=f"ps{mi}")
                        for mi in range(MPG)
                    ]
                    for ko in range(KO):
                        for mi in range(MPG):
                            m0 = (mg * MPG + mi) * P
                            nc.tensor.matmul(
                                psums[mi][:],
                                x_sb[ko][:, m0:m0 + P],
                                w_sb[(nt, ko)][:],
                                start=(ko == 0),
                                stop=(ko == KO - 1),
                            )
                    for mi in range(MPG):
                        m0 = (mg * MPG + mi) * P
                        ot = opool.tile([P, NW], mybir.dt.float32,
                                        name=f"o_{nt}_{mg}_{mi}", tag="o")
                        nc.vector.tensor_add(
                            ot[:], psums[mi][:],
                            bias_sb[:, nt * NW:(nt + 1) * NW])
                        nc.sync.dma_start(
                            out[m0:m0 + P, nt * NW:(nt + 1) * NW], ot[:])

    nc.compile()
    return nc


def _get_nc():
    if "nc" not in _CACHE:
        _CACHE["nc"] = _build_nc()
    return _CACHE["nc"]


def make_in_maps(x, weight, bias, lora_A, lora_B):
    """Host-side shard prep: returns the per-core input maps."""
    w_eff = weight.T.astype(np.float32) + np.float32(SCALING) * (
        lora_A.T.astype(np.float32) @ lora_B.T.astype(np.float32))
    w16 = w_eff.astype(np.float16)
    biasr = np.ascontiguousarray(
        np.broadcast_to(bias.astype(np.float32), (P, OUT_F)))
    xf = np.asarray(x, dtype=np.float32).reshape(M_TOTAL, IN_F)
    in_maps = []
    for c in range(N_CORES):
        xs = xf[c * M_CORE:(c + 1) * M_CORE]
        xT = np.ascontiguousarray(xs.T, dtype=np.float16)
        in_maps.append({"xT": xT, "w": w16, "biasr": biasr})
    return in_maps


def kernel(x, weight, bias, lora_A, lora_B):
    from concourse.bass_utils import run_bass_kernel_spmd

    nc = _get_nc()
    in_maps = make_in_maps(x, weight, bias, lora_A, lora_B)
    res = run_bass_kernel_spmd(nc, in_maps, core_ids=list(range(N_CORES)))
    _CACHE["last_result"] = res
    out = np.concatenate([r["out"] for r in res.results], axis=0)
    return out.reshape(4, 4096, OUT_F)
